# revision 2
# baseline (speedup 1.0000x reference)
"""Trainium2 Bass kernel: 3x3 SAME-padding stride-1 conv2d (NCHW / OIHW).

Full shapes: x (32, 64, 112, 112) f32, kernel (64, 64, 3, 3) f32 -> out (32, 64, 112, 112) f32.

Strategy (data-parallel over batch, 4 images per core on 8 cores):
  Implicit GEMM with full 128x128 PE utilization.
  - M = 128 = 64 out-channels x 2 output-row parities (even/odd rows).
  - K is tiled as 6 chunks of 128 = 64 in-channels x 2 adjacent row-offsets.
    The row-offset pairing is realized by storing TWO copies of the padded
    image in SBUF: partitions 0-63 hold the image at element offset +114
    (one padded row), partitions 64-127 hold it at offset 0. A single 128-
    partition access pattern then reads row r on the low half and row r+1 on
    the high half.
  - Weights are pre-scrambled host-side into 6 [128,128] bf16 chunks; inputs
    are pre-padded (114x114), duplicated and cast to bf16 host-side.
  - Each PSUM chunk [128, 448] covers 8 output rows (4 even + 4 odd) x 112
    cols and accumulates the 6 matmuls in fp32.
"""

import sys

if "/opt/trn_rl_repo" not in sys.path:
    sys.path.insert(0, "/opt/trn_rl_repo")

from contextlib import ExitStack

import ml_dtypes
import numpy as np

import concourse.tile as tile
from concourse import bacc, mybir
from concourse.bass_utils import run_bass_kernel_spmd

N_CORES = 8
IMGS_PER_CORE = 4
C = 64
H = W = 112
WP = 114              # padded width (and padded height)
FLAT = WP * WP        # 12996 elements of one padded image plane
OFF = WP              # copy-1 starts one (padded) row into the buffer
L = 13128             # SBUF free-dim length; >= OFF+FLAT with slack for AP slicing
COMBOS = [(0, 0), (0, 1), (0, 2), (2, 0), (2, 1), (2, 2)]  # (row-offset pair base jr, dx)
CHUNK_YB = 4          # even/odd row pairs per PSUM chunk -> N = 4*112 = 448
N_CHUNK = CHUNK_YB * W
HALF_CHUNKS = 7       # chunks batched into one output staging buffer / DMA
N_CHUNKS = 14         # 14 chunks x 4 row-pairs = 56 pairs = 112 rows

_program_cache = {}


def build_program(n_imgs=IMGS_PER_CORE):
    """Build + compile the per-core Bass program (same program on all cores)."""
    if n_imgs in _program_cache:
        return _program_cache[n_imgs]

    nc = bacc.Bacc(
        "TRN2", target_bir_lowering=False, debug=False, num_devices=N_CORES
    )
    x_dram = nc.dram_tensor(
        "x", [n_imgs, 128, L], mybir.dt.bfloat16, kind="ExternalInput"
    ).ap()
    w_dram = nc.dram_tensor(
        "w2", [128, 6 * 128], mybir.dt.bfloat16, kind="ExternalInput"
    ).ap()
    out_dram = nc.dram_tensor(
        "out", [n_imgs, C, H, W], mybir.dt.float32, kind="ExternalOutput"
    ).ap()

    with tile.TileContext(nc) as tc, ExitStack() as ctx:
        wp = ctx.enter_context(tc.tile_pool(name="wpool", bufs=1))
        xp = ctx.enter_context(tc.tile_pool(name="xpool", bufs=2))
        pp = ctx.enter_context(tc.tile_pool(name="psum", bufs=8, space="PSUM"))
        sp = ctx.enter_context(tc.tile_pool(name="stage", bufs=2))

        w2 = wp.tile([128, 6 * 128], mybir.dt.bfloat16)
        nc.sync.dma_start(w2[:], w_dram[:, :])

        for img in range(n_imgs):
            xt = xp.tile([128, L], mybir.dt.bfloat16)
            nc.sync.dma_start(xt[:], x_dram[img])
            for half in range(2):
                s0 = half * HALF_CHUNKS
                stage = sp.tile([128, HALF_CHUNKS * N_CHUNK], mybir.dt.float32)
                for i in range(HALF_CHUNKS):
                    s = s0 + i
                    yb0 = s * CHUNK_YB
                    psum = pp.tile([128, N_CHUNK], mybir.dt.float32)
                    for wi, (jrp, dx) in enumerate(COMBOS):
                        theta = OFF + (2 * yb0 + jrp) * WP + dx
                        rhs = (
                            xt[:, theta : theta + CHUNK_YB * 2 * WP]
                            .rearrange("p (g w) -> p g w", w=2 * WP)[:, :, :W]
                        )
                        nc.tensor.matmul(
                            psum[:],
                            w2[:, wi * 128 : (wi + 1) * 128],
                            rhs,
                            start=(wi == 0),
                            stop=(wi == len(COMBOS) - 1),
                        )
                    nc.vector.tensor_copy(
                        stage[:, i * N_CHUNK : (i + 1) * N_CHUNK], psum[:]
                    )
                q0 = s0 * CHUNK_YB
                dst = out_dram[img].rearrange("c (q py) x -> py c q x", py=2)[
                    :, :, q0 : q0 + HALF_CHUNKS * CHUNK_YB, :
                ]
                nc.sync.dma_start(dst[0], stage[0:64, :])
                nc.sync.dma_start(dst[1], stage[64:128, :])

    nc.compile()
    _program_cache[n_imgs] = nc
    return nc


def prep_inputs(x, kernel):
    """Host-side preprocessing: pad+duplicate+cast x; scramble weights."""
    n = x.shape[0]
    xpad = np.pad(x, ((0, 0), (0, 0), (1, 1), (1, 1))).astype(ml_dtypes.bfloat16)
    flat = xpad.reshape(n, C, FLAT)
    buf = np.zeros((n, 128, L), ml_dtypes.bfloat16)
    buf[:, :C, OFF : OFF + FLAT] = flat
    buf[:, C:, :FLAT] = flat

    w2 = np.zeros((6, 128, 128), np.float32)
    for wi, (jrp, dx) in enumerate(COMBOS):
        for ph in range(2):        # partition half -> row offset jr = jrp + ph
            jr = jrp + ph
            for py in range(2):    # output row parity
                dy = jr - py
                if 0 <= dy <= 2:
                    w2[wi, ph * 64 : (ph + 1) * 64, py * 64 : (py + 1) * 64] = kernel[
                        :, :, dy, dx
                    ].T
    w2 = (
        w2.transpose(1, 0, 2)
        .reshape(128, 6 * 128)
        .astype(ml_dtypes.bfloat16)
    )
    return buf, w2


def run(x, kernel, trace=False, **trace_kwargs):
    """Run on all 8 cores. Returns (out, BassKernelResults)."""
    buf, w2 = prep_inputs(np.asarray(x, np.float32), np.asarray(kernel, np.float32))
    nc = build_program(IMGS_PER_CORE)
    in_maps = [
        {"x": buf[k * IMGS_PER_CORE : (k + 1) * IMGS_PER_CORE], "w2": w2}
        for k in range(N_CORES)
    ]
    res = run_bass_kernel_spmd(
        nc, in_maps, list(range(N_CORES)), trace=trace, **trace_kwargs
    )
    out = np.concatenate(
        [res.results[k]["out"] for k in range(N_CORES)], axis=0
    ).astype(np.float32)
    return out, res


def kernel(x, kernel):
    out, _ = run(x, kernel)
    return out


# revision 5
# speedup vs baseline: 1.2760x; 1.2760x over previous
"""Trainium2 Bass kernel: 3x3 SAME-padding stride-1 conv2d (NCHW / OIHW).

Full shapes: x (32, 64, 112, 112) f32, kernel (64, 64, 3, 3) f32 -> out (32, 64, 112, 112) f32.

Strategy (data-parallel over batch, 4 images per core on 8 cores):
  Implicit GEMM with full 128x128 PE utilization.
  - M = 128 = 64 out-channels x 2 output-row parities (even/odd rows).
  - K is tiled as 6 chunks of 128 = 64 in-channels x 2 adjacent row-offsets.
    The row-offset pairing is realized by storing TWO copies of the padded
    image in SBUF: partitions 0-63 hold the image at element offset +114
    (one padded row), partitions 64-127 hold it at offset 0. A single 128-
    partition access pattern then reads row r on the low half and row r+1 on
    the high half.
  - Weights are pre-scrambled host-side into 6 [128,128] bf16 chunks; inputs
    are pre-padded (114x114), duplicated and cast to bf16 host-side.
  - Each PSUM chunk [128, 448] covers 8 output rows (4 even + 4 odd) x 112
    cols and accumulates the 6 matmuls in fp32.
"""

import sys

if "/opt/trn_rl_repo" not in sys.path:
    sys.path.insert(0, "/opt/trn_rl_repo")

from contextlib import ExitStack

import ml_dtypes
import numpy as np

import concourse.tile as tile
from concourse import bacc, mybir
from concourse.bass_utils import run_bass_kernel_spmd

N_CORES = 8
IMGS_PER_CORE = 4
C = 64
H = W = 112
WP = 114              # padded width (and padded height)
FLAT = WP * WP        # 12996 elements of one padded image plane
OFF = WP              # copy-1 starts one (padded) row into the buffer
L = 13128             # SBUF free-dim length; >= OFF+FLAT with slack for AP slicing
COMBOS = [(0, 0), (0, 1), (0, 2), (2, 0), (2, 1), (2, 2)]  # (row-offset pair base jr, dx)
CHUNK_YB = 4          # even/odd row pairs per PSUM chunk -> N = 4*112 = 448
N_CHUNK = CHUNK_YB * W
HALF_CHUNKS = 7       # chunks batched into one output staging buffer / DMA
N_CHUNKS = 14         # 14 chunks x 4 row-pairs = 56 pairs = 112 rows

_program_cache = {}


def build_program(n_imgs=IMGS_PER_CORE):
    """Build + compile the per-core Bass program (same program on all cores)."""
    if n_imgs in _program_cache:
        return _program_cache[n_imgs]

    nc = bacc.Bacc(
        "TRN2", target_bir_lowering=False, debug=False, num_devices=N_CORES
    )
    x_dram = nc.dram_tensor(
        "x", [n_imgs, 128, L], mybir.dt.bfloat16, kind="ExternalInput"
    ).ap()
    w_dram = nc.dram_tensor(
        "w2", [128, 6 * 128], mybir.dt.bfloat16, kind="ExternalInput"
    ).ap()
    # Device-friendly output layout: [img, py, c, q, x] (py = row parity,
    # q = row pair). Host interleaves parities back to [c, h, w] at the end.
    out_dram = nc.dram_tensor(
        "out", [n_imgs, 2, C, H // 2, W], mybir.dt.float32, kind="ExternalOutput"
    ).ap()

    with tile.TileContext(nc) as tc, ExitStack() as ctx:
        wp = ctx.enter_context(tc.tile_pool(name="wpool", bufs=1))
        xp = ctx.enter_context(tc.tile_pool(name="xpool", bufs=2))
        pp = ctx.enter_context(tc.tile_pool(name="psum", bufs=8, space="PSUM"))
        sp = ctx.enter_context(tc.tile_pool(name="stage", bufs=2))

        w2 = wp.tile([128, 6 * 128], mybir.dt.bfloat16)
        nc.sync.dma_start(w2[:], w_dram[:, :])

        for img in range(n_imgs):
            xt = xp.tile([128, L], mybir.dt.bfloat16)
            nc.sync.dma_start(xt[:], x_dram[img])
            for half in range(2):
                s0 = half * HALF_CHUNKS
                stage = sp.tile([128, HALF_CHUNKS * N_CHUNK], mybir.dt.float32)
                for i in range(HALF_CHUNKS):
                    s = s0 + i
                    yb0 = s * CHUNK_YB
                    psum = pp.tile([128, N_CHUNK], mybir.dt.float32)
                    for wi, (jrp, dx) in enumerate(COMBOS):
                        theta = OFF + (2 * yb0 + jrp) * WP + dx
                        rhs = (
                            xt[:, theta : theta + CHUNK_YB * 2 * WP]
                            .rearrange("p (g w) -> p g w", w=2 * WP)[:, :, :W]
                        )
                        nc.tensor.matmul(
                            psum[:],
                            w2[:, wi * 128 : (wi + 1) * 128],
                            rhs,
                            start=(wi == 0),
                            stop=(wi == len(COMBOS) - 1),
                        )
                    nc.vector.tensor_copy(
                        stage[:, i * N_CHUNK : (i + 1) * N_CHUNK], psum[:]
                    )
                q0 = s0 * CHUNK_YB
                dst = out_dram[img].rearrange("py c q x -> (py c) (q x)")[
                    :, q0 * W : (q0 + HALF_CHUNKS * CHUNK_YB) * W
                ]
                nc.sync.dma_start(dst, stage[:])

    nc.compile()
    _program_cache[n_imgs] = nc
    return nc


def prep_inputs(x, kernel):
    """Host-side preprocessing: pad+duplicate+cast x; scramble weights."""
    n = x.shape[0]
    xpad = np.pad(x, ((0, 0), (0, 0), (1, 1), (1, 1))).astype(ml_dtypes.bfloat16)
    flat = xpad.reshape(n, C, FLAT)
    buf = np.zeros((n, 128, L), ml_dtypes.bfloat16)
    buf[:, :C, OFF : OFF + FLAT] = flat
    buf[:, C:, :FLAT] = flat

    w2 = np.zeros((6, 128, 128), np.float32)
    for wi, (jrp, dx) in enumerate(COMBOS):
        for ph in range(2):        # partition half -> row offset jr = jrp + ph
            jr = jrp + ph
            for py in range(2):    # output row parity
                dy = jr - py
                if 0 <= dy <= 2:
                    w2[wi, ph * 64 : (ph + 1) * 64, py * 64 : (py + 1) * 64] = kernel[
                        :, :, dy, dx
                    ].T
    w2 = (
        w2.transpose(1, 0, 2)
        .reshape(128, 6 * 128)
        .astype(ml_dtypes.bfloat16)
    )
    return buf, w2


def run(x, kernel, trace=False, **trace_kwargs):
    """Run on all 8 cores. Returns (out, BassKernelResults)."""
    buf, w2 = prep_inputs(np.asarray(x, np.float32), np.asarray(kernel, np.float32))
    nc = build_program(IMGS_PER_CORE)
    in_maps = [
        {"x": buf[k * IMGS_PER_CORE : (k + 1) * IMGS_PER_CORE], "w2": w2}
        for k in range(N_CORES)
    ]
    res = run_bass_kernel_spmd(
        nc, in_maps, list(range(N_CORES)), trace=trace, **trace_kwargs
    )
    out = np.concatenate(
        [res.results[k]["out"] for k in range(N_CORES)], axis=0
    )  # [32, 2, C, H//2, W]
    # interleave row parities: out_full[n, c, 2q+py, x] = out[n, py, c, q, x]
    out = np.ascontiguousarray(out.transpose(0, 2, 3, 1, 4)).reshape(
        x.shape[0], C, H, W
    ).astype(np.float32)
    return out, res


def kernel(x, kernel):
    out, _ = run(x, kernel)
    return out
